# revision 6
# baseline (speedup 1.0000x reference)
"""Delta-accumulation GRU kernel for Trainium2 (8 NeuronCores, no
collectives; data-parallel over batch, 64 rows/core).

Gate pre-activations live in PSUM across all 64 steps:
    S_t = S_{t-1} + d_{t-1} @ W     where d = h_t - h_{t-1}
so steps 3..64 stream only delta matmuls. v2 structure (from NTFF
hardware traces):

  - The per-step critical chain after the matmul phase is only
    z -> zc=sigmoid(-S_z) -> d = zc*(n-h) -> xT = dma_transpose(d):
    the n-path (rs/tt/uu/tanh/vp) is computed DURING the phase because
    its inputs (r, ghn, gin chunks) stream before the z chunks.
  - xT is produced by the hardware xbar DMA transpose (HWDGE,
    SBUF->SBUF, bf16) directly into SBUF as [128, 4, 64] k-tiles --
    no PE transposes, no PSUM staging, no assemble copies.
  - dd/xT are split into 512-column halves on separate HWDGE rings
    (sync + scalar) so half1 transposes while half0's matmuls start.
  - h, n, d are bf16: DVE 2x modes and half-size output DMA
    (out is written bf16 and upcast to f32 on the host).
  - Startup: ctxT+wctx on the sync ring (h0 matmuls chase wctx
    chunks), whh on the scalar ring, wall on the gpsimd ring, so the
    21MB of weights stream concurrently and steps 1-2 chase them.

Column groups (two concurrent PE tile_position streams):
    group0 (tile_position (0,0), psum parts 0-63):   r0 r1 gin0 z1
    group1 (tile_position (0,64), psum parts 64-127): ghn0 ghn1 gin1 z0

PSUM (8 banks, all persistent):
    pA [128,1024]: r (parts 0-63)
    pB [128,1024]: gin0 | z1 (parts 0-63)
    pC [128,1024]: ghn (parts 64-127)
    pD [128,1024]: gin1 | z0 (parts 64-127)
"""

import numpy as np
import ml_dtypes

import concourse.bass as bass
import concourse.bacc as bacc
import concourse.mybir as mybir
import concourse.tile as tile
from concourse.bass_utils import run_bass_kernel_spmd

BF16 = mybir.dt.bfloat16
F32 = mybir.dt.float32
AF = mybir.ActivationFunctionType

B, D, T = 512, 1024, 64
NCORES = 8
BL = B // NCORES
CTX = 3072
NK = D // 128
NKC = CTX // 128
CH = 512
NKH = NK // 2  # k-tiles per half

_CACHE = {}
TRACE = False
TRACE_KW = {}
LAST_RESULT = [None]
LAST_IN_MAPS = [None]


def _build_nc(n_steps=T):
    nc = bacc.Bacc("TRN2")

    ctxT_h = nc.declare_dram_parameter("ctxT", [CTX, BL], BF16, isOutput=False)
    wctx_h = nc.declare_dram_parameter("wctx", [CTX, D], BF16, isOutput=False)
    whh_h = nc.declare_dram_parameter("whh", [D, 3 * D], BF16, isOutput=False)
    wall_h = nc.declare_dram_parameter("wall", [D, 4 * D], BF16, isOutput=False)
    bctx_h = nc.declare_dram_parameter("bctx", [1, D], BF16, isOutput=False)
    bias1_h = nc.declare_dram_parameter("bias1", [1, 3 * D], BF16, isOutput=False)
    gin1_h = nc.declare_dram_parameter("gin1", [1, D], F32, isOutput=False)
    biasM_h = nc.declare_dram_parameter("biasM", [1, 4 * D], BF16, isOutput=False)
    out_h = nc.declare_dram_parameter("out", [T, BL, D], BF16, isOutput=True)

    with tile.TileContext(nc) as tc:
        with (
            tc.tile_pool(name="wres", bufs=1) as wres,
            tc.tile_pool(name="wstream", bufs=3) as wstream,
            tc.tile_pool(name="consts", bufs=1) as consts,
            tc.tile_pool(name="state", bufs=2) as state,
            tc.tile_pool(name="work", bufs=1) as work,
            tc.tile_pool(name="psum", bufs=1, space="PSUM") as psum,
        ):
            # ---- startup DMAs, split across the three DGE rings ----
            # sync ring: ctxT then wctx chunks (h0 matmuls chase them)
            ctxT_sb = consts.tile([128, NKC, BL], BF16)
            nc.sync.dma_start(
                out=ctxT_sb, in_=ctxT_h[:].rearrange("(ko p) b -> p ko b", p=128)
            )
            wctx_t = wctx_h[:].rearrange("(ko p) n -> p ko n", p=128)
            # scalar ring: whh (step 1 chases)
            whh_sb = wres.tile([128, NK, 3 * D], BF16, tag="whh")
            whh_t = whh_h[:].rearrange("(ko p) n -> p ko n", p=128)
            for q in range(4):
                nc.scalar.dma_start(
                    out=whh_sb[:, 2 * q : 2 * q + 2, :],
                    in_=whh_t[:, 2 * q : 2 * q + 2, :],
                )
            # gpsimd ring: wall (steps 2+ chase) + small fry
            wall_sb = wres.tile([128, NK, 4 * D], BF16, tag="wall")
            wall_t = wall_h[:].rearrange("(ko p) n -> p ko n", p=128)
            for q in range(4):
                nc.gpsimd.dma_start(
                    out=wall_sb[:, 2 * q : 2 * q + 2, :],
                    in_=wall_t[:, 2 * q : 2 * q + 2, :],
                )
            bctx_sb = consts.tile([1, D], BF16)
            nc.sync.dma_start(out=bctx_sb, in_=bctx_h[:])
            bias1_sb = consts.tile([1, 3 * D], BF16)
            nc.sync.dma_start(out=bias1_sb, in_=bias1_h[:])
            biasM_sb = consts.tile([1, 4 * D], BF16)
            nc.sync.dma_start(out=biasM_sb, in_=biasM_h[:])
            gin1_bc = consts.tile([BL, D], F32)
            g1 = gin1_h[:]
            g1_bc = bass.AP(tensor=g1.tensor, offset=g1.offset, ap=[[0, BL], [1, D]])
            nc.gpsimd.dma_start(out=gin1_bc, in_=g1_bc)
            ones_sb = consts.tile([1, BL], BF16)
            nc.vector.memset(ones_sb, 1.0)

            # persistent PSUM accumulators
            pA = psum.tile([128, D], F32, tag="pA")
            pB = psum.tile([128, D], F32, tag="pB")
            pC = psum.tile([128, D], F32, tag="pC")
            pD = psum.tile([128, D], F32, tag="pD")
            r_ap = pA[0:64, :]
            gin0_ap = pB[0:64, 0:CH]
            z1_ap = pB[0:64, CH:D]
            ghn_ap = pC[64:128, :]
            gin1_ap = pD[64:128, 0:CH]
            z0_ap = pD[64:128, CH:D]

            def transpose_to(src0, src1, i):
                """bf16 [64,512] halves -> xT half tiles [128,4,64] via the
                hardware xbar DMA transpose, on separate HWDGE rings."""
                xT0 = state.tile([128, NKH, BL], BF16, tag="xT0", bufs=2,
                                 name=f"xT0_{i}")
                xT1 = state.tile([128, NKH, BL], BF16, tag="xT1", bufs=2,
                                 name=f"xT1_{i}")
                nc.sync.dma_start_transpose(out=xT0, in_=src0)
                nc.scalar.dma_start_transpose(out=xT1, in_=src1)
                return (xT0, xT1)

            # (psum_ap, wcol, tile_position) chunk specs per step kind
            def chunks_steady(gin_c, ghn_c):
                g0 = [(r_ap[:, 0:CH], 0, (0, 0)),
                      (r_ap[:, CH:D], CH, (0, 0)),
                      (gin0_ap, gin_c, (0, 0)),
                      (z1_ap, 3 * CH, (0, 0))]
                g1 = [(ghn_ap[:, 0:CH], ghn_c, (0, 64)),
                      (ghn_ap[:, CH:D], ghn_c + CH, (0, 64)),
                      (gin1_ap, gin_c + CH, (0, 64)),
                      (z0_ap, 2 * CH, (0, 64))]
                return list(zip(g0, g1))

            def chunks_step1(ghn_c):
                g0 = [(r_ap[:, 0:CH], 0, (0, 0)),
                      (r_ap[:, CH:D], CH, (0, 0)),
                      (z1_ap, 3 * CH, (0, 0))]
                g1 = [(ghn_ap[:, 0:CH], ghn_c, (0, 64)),
                      (ghn_ap[:, CH:D], ghn_c + CH, (0, 64)),
                      (z0_ap, 2 * CH, (0, 64))]
                return list(zip(g0, g1))

            def mm_phase(pairs, xTs, w_sb, bias_sb, full):
                xT0, xT1 = xTs
                if full:
                    for pair in pairs:
                        for pap, wcol, tp in pair:
                            nc.tensor.matmul(
                                pap, ones_sb[0:1, :],
                                bias_sb[0:1, wcol : wcol + CH],
                                start=True, stop=False, tile_position=tp,
                            )
                for pair in pairs:
                    for k in range(NK):
                        xT = xT0 if k < NKH else xT1
                        for pap, wcol, tp in pair:
                            nc.tensor.matmul(
                                pap, xT[:, k % NKH, :],
                                w_sb[:, k, wcol : wcol + CH],
                                start=False, stop=(k == NK - 1),
                                tile_position=tp,
                            )

            def tail(s, hprev, step1, last):
                """hprev = (hn0, hn1) bf16 [64,512] tiles."""
                i = nc.next_id()
                hp0, hp1 = hprev
                # n-path: all inputs stream before the z chunks, so these
                # overlap the matmul phase
                rs0 = work.tile([BL, CH], BF16, tag="rs0", name=f"rs0_{i}")
                rs1 = work.tile([BL, CH], BF16, tag="rs1", name=f"rs1_{i}")
                nc.scalar.activation(rs0, r_ap[:, 0:CH], AF.Sigmoid)
                nc.scalar.activation(rs1, r_ap[:, CH:D], AF.Sigmoid)
                tt0 = work.tile([BL, CH], F32, tag="tt0", name=f"tt0_{i}")
                tt1 = work.tile([BL, CH], F32, tag="tt1", name=f"tt1_{i}")
                nc.vector.tensor_mul(tt0, rs0, ghn_ap[:, 0:CH])
                nc.vector.tensor_mul(tt1, rs1, ghn_ap[:, CH:D])
                uu0 = work.tile([BL, CH], F32, tag="uu0", name=f"uu0_{i}")
                uu1 = work.tile([BL, CH], F32, tag="uu1", name=f"uu1_{i}")
                if step1:
                    nc.vector.tensor_add(uu0, tt0, gin1_bc[:, 0:CH])
                    nc.vector.tensor_add(uu1, tt1, gin1_bc[:, CH:D])
                else:
                    nc.vector.tensor_add(uu0, tt0, gin0_ap)
                    nc.vector.tensor_add(uu1, tt1, gin1_ap)
                nn0 = work.tile([BL, CH], BF16, tag="nn0", name=f"nn0_{i}")
                nn1 = work.tile([BL, CH], BF16, tag="nn1", name=f"nn1_{i}")
                nc.scalar.activation(nn0, uu0, AF.Tanh)
                nc.scalar.activation(nn1, uu1, AF.Tanh)
                vp0 = work.tile([BL, CH], BF16, tag="vp0", name=f"vp0_{i}")
                vp1 = work.tile([BL, CH], BF16, tag="vp1", name=f"vp1_{i}")
                nc.vector.tensor_sub(vp0, nn0, hp0)  # v' = n - h
                nc.vector.tensor_sub(vp1, nn1, hp1)
                # critical chain: z -> zc -> dd -> dma transpose
                # zc = sigmoid(-S_z) = 1-z, so d = zc * (n-h) = h'-h
                zc0 = work.tile([BL, CH], BF16, tag="zc0", name=f"zc0_{i}")
                zc1 = work.tile([BL, CH], BF16, tag="zc1", name=f"zc1_{i}")
                nc.scalar.activation(zc0, z0_ap, AF.Sigmoid, scale=-1.0)
                nc.scalar.activation(zc1, z1_ap, AF.Sigmoid, scale=-1.0)
                dd0 = work.tile([BL, CH], BF16, tag="dd0", bufs=2,
                                name=f"dd0_{i}")
                dd1 = work.tile([BL, CH], BF16, tag="dd1", bufs=2,
                                name=f"dd1_{i}")
                nc.vector.tensor_mul(dd0, zc0, vp0)
                nc.vector.tensor_mul(dd1, zc1, vp1)
                hn0 = state.tile([BL, CH], BF16, bufs=3, tag="h0",
                                 name=f"h0_{i}")
                hn1 = state.tile([BL, CH], BF16, bufs=3, tag="h1",
                                 name=f"h1_{i}")
                if last:
                    nc.vector.tensor_add(hn0, hp0, dd0)
                    nc.vector.tensor_add(hn1, hp1, dd1)
                    nc.gpsimd.dma_start(out=out_h[s][:, 0:CH], in_=hn0)
                    nc.gpsimd.dma_start(out=out_h[s][:, CH:D], in_=hn1)
                    return (hn0, hn1), None
                if step1:
                    # step 2 is a full write S_2 = bias + h1 @ W_all, so it
                    # needs h1 transposed, not the delta
                    nc.vector.tensor_add(hn0, hp0, dd0)
                    nc.vector.tensor_add(hn1, hp1, dd1)
                    xTs = transpose_to(hn0, hn1, i)
                    nc.gpsimd.dma_start(out=out_h[s][:, 0:CH], in_=hn0)
                    nc.gpsimd.dma_start(out=out_h[s][:, CH:D], in_=hn1)
                    return (hn0, hn1), xTs
                xTs = transpose_to(dd0, dd1, i)
                # h' update and output DMA are off the critical path
                nc.vector.tensor_add(hn0, hp0, dd0)
                nc.vector.tensor_add(hn1, hp1, dd1)
                nc.gpsimd.dma_start(out=out_h[s][:, 0:CH], in_=hn0)
                nc.gpsimd.dma_start(out=out_h[s][:, CH:D], in_=hn1)
                return (hn0, hn1), xTs

            # ---- h0 (into pA parts 0-63, before step 1 overwrites) ----
            ph0 = pA[0:64, :]
            for c in range(2):
                nc.tensor.matmul(
                    ph0[:, c * CH : (c + 1) * CH], ones_sb[0:1, :],
                    bctx_sb[0:1, c * CH : (c + 1) * CH],
                    start=True, stop=False, tile_position=(0, 0),
                )
            for q in range(NKC // 2):
                wk = wstream.tile([128, 2, D], BF16, tag="wctxk", name=f"wk_{q}")
                nc.sync.dma_start(out=wk, in_=wctx_t[:, 2 * q : 2 * q + 2, :])
                for j in range(2):
                    kc = 2 * q + j
                    for c in range(2):
                        nc.tensor.matmul(
                            ph0[:, c * CH : (c + 1) * CH], ctxT_sb[:, kc, :],
                            wk[:, j, c * CH : (c + 1) * CH],
                            start=False, stop=(kc == NKC - 1),
                            tile_position=(0, 0),
                        )
            h00 = state.tile([BL, CH], BF16, tag="h0", bufs=3)
            h01 = state.tile([BL, CH], BF16, tag="h1", bufs=3)
            nc.vector.tensor_copy(h00, ph0[:, 0:CH])
            nc.vector.tensor_copy(h01, ph0[:, CH:D])
            hT0 = transpose_to(h00, h01, 0)

            # ---- step 1: full write, W_hh ----
            mm_phase(chunks_step1(2 * D), hT0, whh_sb, bias1_sb, full=True)
            hprev, xTs = tail(0, (h00, h01), step1=True, last=(n_steps == 1))

            # ---- step 2: full write, W_all ----
            if n_steps >= 2:
                mm_phase(chunks_steady(2 * D, 3 * D), xTs, wall_sb, biasM_sb,
                         full=True)
                hprev, xTs = tail(1, hprev, step1=False, last=(n_steps == 2))

            # ---- steps 3..n: accumulate deltas ----
            # (n_steps > T is a timing-only mode: out index wraps)
            for s in range(2, n_steps):
                mm_phase(chunks_steady(2 * D, 3 * D), xTs, wall_sb, biasM_sb,
                         full=False)
                hprev, xTs = tail(s % T, hprev, step1=False,
                                  last=(s == n_steps - 1))

    nc.finalize()
    return nc


def kernel(world_state, goal, W_ctx, b_ctx, start_token, W_ih, b_ih, W_hh, b_hh):
    bf16 = ml_dtypes.bfloat16
    ws = np.asarray(world_state, dtype=np.float32)
    gl = np.asarray(goal, dtype=np.float32)
    W_ctx = np.asarray(W_ctx, dtype=np.float32)
    b_ctx = np.asarray(b_ctx, dtype=np.float32)
    start_token = np.asarray(start_token, dtype=np.float32)
    W_ih = np.asarray(W_ih, dtype=np.float32)
    b_ih = np.asarray(b_ih, dtype=np.float32)
    W_hh = np.asarray(W_hh, dtype=np.float32)
    b_hh = np.asarray(b_hh, dtype=np.float32)

    if "nc" not in _CACHE:
        _CACHE["nc"] = _build_nc()
    nc = _CACHE["nc"]

    ctxT = np.ascontiguousarray(np.concatenate([ws, gl], axis=1).T)
    ctxT_bf = ctxT.astype(bf16)
    wctx_bf = np.ascontiguousarray(W_ctx).astype(bf16)
    whh_bf = np.ascontiguousarray(W_hh).astype(bf16)
    wall_bf = np.ascontiguousarray(
        np.concatenate(
            [W_ih[:, : 2 * D] + W_hh[:, : 2 * D], W_ih[:, 2 * D :], W_hh[:, 2 * D :]],
            axis=1,
        )
    ).astype(bf16)
    gi1 = start_token @ W_ih + b_ih
    bias1 = np.ascontiguousarray(
        np.concatenate([gi1[: 2 * D] + b_hh[: 2 * D], b_hh[2 * D :]])
    ).astype(bf16)[None]
    gin1 = np.ascontiguousarray(gi1[2 * D :].astype(np.float32))[None]
    biasM = np.ascontiguousarray(
        np.concatenate([b_ih[: 2 * D] + b_hh[: 2 * D], b_ih[2 * D :], b_hh[2 * D :]])
    ).astype(bf16)[None]
    bctx = np.ascontiguousarray(b_ctx).astype(bf16)[None]

    shared = dict(
        wctx=wctx_bf, whh=whh_bf, wall=wall_bf, bctx=bctx,
        bias1=bias1, gin1=gin1, biasM=biasM,
    )
    in_maps = [
        {**shared, "ctxT": np.ascontiguousarray(ctxT_bf[:, i * BL : (i + 1) * BL])}
        for i in range(NCORES)
    ]

    LAST_IN_MAPS[0] = in_maps
    res = run_bass_kernel_spmd(
        nc, in_maps, core_ids=list(range(NCORES)), trace=TRACE, **TRACE_KW
    )
    LAST_RESULT[0] = res

    full = np.empty((B, T, D), dtype=np.float32)
    for i in range(NCORES):
        o = np.asarray(res.results[i]["out"]).astype(np.float32)
        full[i * BL : (i + 1) * BL] = o.transpose(1, 0, 2)
    return full


# revision 22
# speedup vs baseline: 1.1454x; 1.1454x over previous
"""Delta-accumulation GRU kernel for Trainium2 (8 NeuronCores, no
collectives; data-parallel over batch, 64 rows/core).

Gate pre-activations live in PSUM across all 64 steps:
    S_t = S_{t-1} + d_{t-1} @ W     where d = h_t - h_{t-1}
so steps 3..64 stream only delta matmuls. Structure (from NTFF
hardware traces; ~1.10 ms total, ~15.5 us/step, vs 1.16 ms for the
f32-h PE-transpose v1):

  - The per-step critical chain after the matmul phase is only
    z -> zc=sigmoid(-S_z) -> d = zc*(n-h) -> xT = dma_transpose(d):
    the n-path (rs/tt/uu/tanh/vp) is computed DURING the phase because
    its inputs (r, ghn, gin chunks) stream before the z chunks.
  - xT is produced by the hardware xbar DMA transpose (HWDGE,
    SBUF->SBUF, bf16) directly into SBUF as [128, 4, 64] k-tiles --
    no PE transposes, no PSUM staging, no assemble copies.  (PE
    transposes + PSUM staging copies were tried in several layouts:
    they leave the phase at the HAM-throttled 1.2 GHz clock for
    3-6 us/step -- net worse.  fp8 DoubleRow matmuls would halve the
    phase but fail accuracy by 4-8x: bf16 is the precision floor.)
  - dd/xT are split into 512-column halves on separate HWDGE rings
    (sync + scalar) so half1 transposes while half0's matmuls start;
    on the strict-FIFO DVE, dd0 is issued before vp1 so it is not
    stuck behind tanh1's dependency.
  - h, n, d are bf16: DVE 2x modes and half-size output DMA
    (out is written bf16 and upcast to f32 on the host); rel err
    0.0112 vs the f32 reference (gate 2e-2).
  - Startup: biases first then ctxT+wctx on the sync ring (h0
    matmuls chase wctx chunks), whh on the scalar ring, wall on the
    gpsimd ring, so the 21MB of weights stream concurrently and
    steps 1-2 chase them.

Column groups (two concurrent PE tile_position streams):
    group0 (tile_position (0,0), psum parts 0-63):   r0 r1 gin0 z1
    group1 (tile_position (0,64), psum parts 64-127): ghn0 ghn1 gin1 z0

PSUM (8 banks, all persistent):
    pA [128,1024]: r (parts 0-63)
    pB [128,1024]: gin0 | z1 (parts 0-63)
    pC [128,1024]: ghn (parts 64-127)
    pD [128,1024]: gin1 | z0 (parts 64-127)
"""

import numpy as np
import ml_dtypes

import concourse.bass as bass
import concourse.bacc as bacc
import concourse.mybir as mybir
import concourse.tile as tile
from concourse.bass_utils import run_bass_kernel_spmd
from concourse.masks import make_identity

BF16 = mybir.dt.bfloat16
F32 = mybir.dt.float32
AF = mybir.ActivationFunctionType

B, D, T = 512, 1024, 64
NCORES = 8
BL = B // NCORES
CTX = 3072
NK = D // 128
NKC = CTX // 128
CH = 512
NKH = NK // 2  # k-tiles per half

_CACHE = {}
TRACE = False
TRACE_KW = {}
LAST_RESULT = [None]
LAST_IN_MAPS = [None]


def _build_nc(n_steps=T):
    nc = bacc.Bacc("TRN2")

    ctxT_h = nc.declare_dram_parameter("ctxT", [CTX, BL], BF16, isOutput=False)
    wctx_h = nc.declare_dram_parameter("wctx", [CTX, D], BF16, isOutput=False)
    whh_h = nc.declare_dram_parameter("whh", [D, 3 * D], BF16, isOutput=False)
    wall_h = nc.declare_dram_parameter("wall", [D, 4 * D], BF16, isOutput=False)
    bctx_h = nc.declare_dram_parameter("bctx", [1, D], BF16, isOutput=False)
    bias1_h = nc.declare_dram_parameter("bias1", [1, 3 * D], BF16, isOutput=False)
    gin1_h = nc.declare_dram_parameter("gin1", [1, D], F32, isOutput=False)
    biasM_h = nc.declare_dram_parameter("biasM", [1, 4 * D], BF16, isOutput=False)
    out_h = nc.declare_dram_parameter("out", [T, BL, D], BF16, isOutput=True)

    with tile.TileContext(nc) as tc:
        with (
            tc.tile_pool(name="wres", bufs=1) as wres,
            tc.tile_pool(name="wstream", bufs=3) as wstream,
            tc.tile_pool(name="consts", bufs=1) as consts,
            tc.tile_pool(name="state", bufs=2) as state,
            tc.tile_pool(name="work", bufs=1) as work,
            tc.tile_pool(name="psum", bufs=1, space="PSUM") as psum,
        ):
            # ---- startup DMAs, split across the three DGE rings ----
            # sync ring: small biases FIRST (h0's bias-row matmuls need
            # them), then ctxT, then wctx chunks (h0 matmuls chase them)
            bctx_sb = consts.tile([1, D], BF16)
            nc.sync.dma_start(out=bctx_sb, in_=bctx_h[:])
            bias1_sb = consts.tile([1, 3 * D], BF16)
            nc.sync.dma_start(out=bias1_sb, in_=bias1_h[:])
            biasM_sb = consts.tile([1, 4 * D], BF16)
            nc.sync.dma_start(out=biasM_sb, in_=biasM_h[:])
            ctxT_sb = consts.tile([128, NKC, BL], BF16)
            nc.sync.dma_start(
                out=ctxT_sb, in_=ctxT_h[:].rearrange("(ko p) b -> p ko b", p=128)
            )
            wctx_t = wctx_h[:].rearrange("(ko p) n -> p ko n", p=128)
            # scalar ring: whh (step 1 chases)
            whh_sb = wres.tile([128, NK, 3 * D], BF16, tag="whh")
            whh_t = whh_h[:].rearrange("(ko p) n -> p ko n", p=128)
            for q in range(4):
                nc.scalar.dma_start(
                    out=whh_sb[:, 2 * q : 2 * q + 2, :],
                    in_=whh_t[:, 2 * q : 2 * q + 2, :],
                )
            # gpsimd ring: wall (steps 2+ chase) + small fry
            wall_sb = wres.tile([128, NK, 4 * D], BF16, tag="wall")
            wall_t = wall_h[:].rearrange("(ko p) n -> p ko n", p=128)
            for q in range(4):
                nc.gpsimd.dma_start(
                    out=wall_sb[:, 2 * q : 2 * q + 2, :],
                    in_=wall_t[:, 2 * q : 2 * q + 2, :],
                )
            gin1_bc = consts.tile([BL, D], F32)
            g1 = gin1_h[:]
            g1_bc = bass.AP(tensor=g1.tensor, offset=g1.offset, ap=[[0, BL], [1, D]])
            nc.gpsimd.dma_start(out=gin1_bc, in_=g1_bc)
            ones_sb = consts.tile([1, BL], BF16)
            nc.vector.memset(ones_sb, 1.0)
            ident_bf = consts.tile([BL, BL], BF16)
            make_identity(nc, ident_bf)

            # persistent PSUM accumulators
            pA = psum.tile([128, D], F32, tag="pA")
            pB = psum.tile([128, D], F32, tag="pB")
            pC = psum.tile([128, D], F32, tag="pC")
            pD = psum.tile([128, D], F32, tag="pD")
            r_ap = pA[0:64, :]
            gin0_ap = pB[0:64, 0:CH]
            z1_ap = pB[0:64, CH:D]
            ghn_ap = pC[64:128, :]
            gin1_ap = pD[64:128, 0:CH]
            z0_ap = pD[64:128, CH:D]
            # transpose staging at parts 0-63 of pC's banks (gates live at
            # parts 64-127 there; has_written bits are per-partition so the
            # start=True staging writes don't disturb the accumulation)
            stage_bf = pC[0:64, :].bitcast(BF16)  # [64, 2048] bf16 view
            stg = stage_bf[:, 0 : 16 * BL].rearrange(
                "p (k two j) -> p k two j", two=2, j=BL
            )

            def transpose_to(src0, src1, i):
                """bf16 [64,512] halves -> xT half tiles [128,4,64] via the
                hardware xbar DMA transpose, on separate HWDGE rings."""
                xT0 = state.tile([128, NKH, BL], BF16, tag="xT0", bufs=2,
                                 name=f"xT0_{i}")
                xT1 = state.tile([128, NKH, BL], BF16, tag="xT1", bufs=2,
                                 name=f"xT1_{i}")
                nc.sync.dma_start_transpose(out=xT0, in_=src0)
                nc.scalar.dma_start_transpose(out=xT1, in_=src1)
                return (xT0, xT1)

            # (psum_ap, wcol, tile_position) chunk specs per step kind
            def chunks_steady(gin_c, ghn_c):
                g0 = [(r_ap[:, 0:CH], 0, (0, 0)),
                      (r_ap[:, CH:D], CH, (0, 0)),
                      (gin0_ap, gin_c, (0, 0)),
                      (z1_ap, 3 * CH, (0, 0))]
                g1 = [(ghn_ap[:, 0:CH], ghn_c, (0, 64)),
                      (ghn_ap[:, CH:D], ghn_c + CH, (0, 64)),
                      (gin1_ap, gin_c + CH, (0, 64)),
                      (z0_ap, 2 * CH, (0, 64))]
                return list(zip(g0, g1))

            def chunks_step1(ghn_c):
                g0 = [(r_ap[:, 0:CH], 0, (0, 0)),
                      (r_ap[:, CH:D], CH, (0, 0)),
                      (z1_ap, 3 * CH, (0, 0))]
                g1 = [(ghn_ap[:, 0:CH], ghn_c, (0, 64)),
                      (ghn_ap[:, CH:D], ghn_c + CH, (0, 64)),
                      (z0_ap, 2 * CH, (0, 64))]
                return list(zip(g0, g1))

            def mm_phase(pairs, xTs, w_sb, bias_sb, full):
                xT0, xT1 = xTs
                if full:
                    for pair in pairs:
                        for pap, wcol, tp in pair:
                            nc.tensor.matmul(
                                pap, ones_sb[0:1, :],
                                bias_sb[0:1, wcol : wcol + CH],
                                start=True, stop=False, tile_position=tp,
                            )
                slots = [(pair, k) for pair in pairs for k in range(NK)]
                for pair, k in slots:
                    xT = xT0 if k < NKH else xT1
                    for pap, wcol, tp in pair:
                        nc.tensor.matmul(
                            pap, xT[:, k % NKH, :],
                            w_sb[:, k, wcol : wcol + CH],
                            start=False, stop=(k == NK - 1),
                            tile_position=tp,
                        )

            def tail(s, hprev, step1, last):
                """hprev = (hn0, hn1) bf16 [64,512] tiles."""
                i = nc.next_id()
                hp0, hp1 = hprev
                # n-path: all inputs stream before the z chunks, so these
                # overlap the matmul phase
                rs0 = work.tile([BL, CH], BF16, tag="rs0", name=f"rs0_{i}")
                rs1 = work.tile([BL, CH], BF16, tag="rs1", name=f"rs1_{i}")
                nc.scalar.activation(rs0, r_ap[:, 0:CH], AF.Sigmoid)
                nc.scalar.activation(rs1, r_ap[:, CH:D], AF.Sigmoid)

                tt0 = work.tile([BL, CH], F32, tag="tt0", name=f"tt0_{i}")
                tt1 = work.tile([BL, CH], F32, tag="tt1", name=f"tt1_{i}")
                nc.vector.tensor_mul(tt0, rs0, ghn_ap[:, 0:CH])
                nc.vector.tensor_mul(tt1, rs1, ghn_ap[:, CH:D])
                uu0 = work.tile([BL, CH], F32, tag="uu0", name=f"uu0_{i}")
                uu1 = work.tile([BL, CH], F32, tag="uu1", name=f"uu1_{i}")
                if step1:
                    nc.vector.tensor_add(uu0, tt0, gin1_bc[:, 0:CH])
                    nc.vector.tensor_add(uu1, tt1, gin1_bc[:, CH:D])
                else:
                    nc.vector.tensor_add(uu0, tt0, gin0_ap)
                    nc.vector.tensor_add(uu1, tt1, gin1_ap)
                nn0 = work.tile([BL, CH], BF16, tag="nn0", name=f"nn0_{i}")
                nn1 = work.tile([BL, CH], BF16, tag="nn1", name=f"nn1_{i}")
                nc.scalar.activation(nn0, uu0, AF.Tanh)
                nc.scalar.activation(nn1, uu1, AF.Tanh)
                vp0 = work.tile([BL, CH], BF16, tag="vp0", name=f"vp0_{i}")
                vp1 = work.tile([BL, CH], BF16, tag="vp1", name=f"vp1_{i}")
                # critical chain: z -> zc -> dd -> dma transpose
                # zc = sigmoid(-S_z) = 1-z, so d = zc * (n-h) = h'-h
                zc0 = work.tile([BL, CH], BF16, tag="zc0", name=f"zc0_{i}")
                zc1 = work.tile([BL, CH], BF16, tag="zc1", name=f"zc1_{i}")
                nc.scalar.activation(zc0, z0_ap, AF.Sigmoid, scale=-1.0)
                nc.scalar.activation(zc1, z1_ap, AF.Sigmoid, scale=-1.0)
                dd0 = work.tile([BL, CH], BF16, tag="dd0", bufs=2,
                                name=f"dd0_{i}")
                dd1 = work.tile([BL, CH], BF16, tag="dd1", bufs=2,
                                name=f"dd1_{i}")
                # DVE order vp0, dd0, vp1, dd1: dd0 must not queue behind
                # vp1 (which waits on tanh1) on the strict-FIFO DVE
                nc.vector.tensor_sub(vp0, nn0, hp0)  # v' = n - h
                nc.vector.tensor_mul(dd0, zc0, vp0)
                nc.vector.tensor_sub(vp1, nn1, hp1)
                nc.vector.tensor_mul(dd1, zc1, vp1)
                hn0 = state.tile([BL, CH], BF16, bufs=3, tag="h0",
                                 name=f"h0_{i}")
                hn1 = state.tile([BL, CH], BF16, bufs=3, tag="h1",
                                 name=f"h1_{i}")
                if last:
                    nc.vector.tensor_add(hn0, hp0, dd0)
                    nc.vector.tensor_add(hn1, hp1, dd1)
                    nc.gpsimd.dma_start(out=out_h[s][:, 0:CH], in_=hn0)
                    nc.gpsimd.dma_start(out=out_h[s][:, CH:D], in_=hn1)
                    return (hn0, hn1), None
                if step1:
                    # step 2 is a full write S_2 = bias + h1 @ W_all, so it
                    # needs h1 transposed, not the delta
                    nc.vector.tensor_add(hn0, hp0, dd0)
                    nc.vector.tensor_add(hn1, hp1, dd1)
                    xTs = transpose_to(hn0, hn1, i)
                    nc.gpsimd.dma_start(out=out_h[s][:, 0:CH], in_=hn0)
                    nc.gpsimd.dma_start(out=out_h[s][:, CH:D], in_=hn1)
                    return (hn0, hn1), xTs
                xTs = transpose_to(dd0, dd1, i)
                # h' update and output DMA are off the critical path
                nc.vector.tensor_add(hn0, hp0, dd0)
                nc.vector.tensor_add(hn1, hp1, dd1)
                nc.gpsimd.dma_start(out=out_h[s][:, 0:CH], in_=hn0)
                nc.gpsimd.dma_start(out=out_h[s][:, CH:D], in_=hn1)
                return (hn0, hn1), xTs

            # ---- h0 (into pA parts 0-63, before step 1 overwrites) ----
            ph0 = pA[0:64, :]
            for c in range(2):
                nc.tensor.matmul(
                    ph0[:, c * CH : (c + 1) * CH], ones_sb[0:1, :],
                    bctx_sb[0:1, c * CH : (c + 1) * CH],
                    start=True, stop=False, tile_position=(0, 0),
                )
            for q in range(NKC // 2):
                wk = wstream.tile([128, 2, D], BF16, tag="wctxk", name=f"wk_{q}")
                nc.sync.dma_start(out=wk, in_=wctx_t[:, 2 * q : 2 * q + 2, :])
                for j in range(2):
                    kc = 2 * q + j
                    for c in range(2):
                        nc.tensor.matmul(
                            ph0[:, c * CH : (c + 1) * CH], ctxT_sb[:, kc, :],
                            wk[:, j, c * CH : (c + 1) * CH],
                            start=False, stop=(kc == NKC - 1),
                            tile_position=(0, 0),
                        )
            h00 = state.tile([BL, CH], BF16, tag="h0", bufs=3)
            h01 = state.tile([BL, CH], BF16, tag="h1", bufs=3)
            nc.vector.tensor_copy(h00, ph0[:, 0:CH])
            nc.vector.tensor_copy(h01, ph0[:, CH:D])
            hT0 = transpose_to(h00, h01, 0)

            # ---- step 1: full write, W_hh ----
            mm_phase(chunks_step1(2 * D), hT0, whh_sb, bias1_sb, full=True)
            hprev, xTs = tail(0, (h00, h01), step1=True, last=(n_steps == 1))

            # ---- step 2: full write, W_all ----
            if n_steps >= 2:
                mm_phase(chunks_steady(2 * D, 3 * D), xTs, wall_sb, biasM_sb,
                         full=True)
                hprev, xTs = tail(1, hprev, step1=False, last=(n_steps == 2))

            # ---- steps 3..n: accumulate deltas ----
            # (n_steps > T is a timing-only mode: out index wraps)
            for s in range(2, n_steps):
                mm_phase(chunks_steady(2 * D, 3 * D), xTs, wall_sb, biasM_sb,
                         full=False)
                hprev, xTs = tail(s % T, hprev, step1=False,
                                  last=(s == n_steps - 1))

    nc.finalize()
    return nc


def kernel(world_state, goal, W_ctx, b_ctx, start_token, W_ih, b_ih, W_hh, b_hh):
    bf16 = ml_dtypes.bfloat16
    ws = np.asarray(world_state, dtype=np.float32)
    gl = np.asarray(goal, dtype=np.float32)
    W_ctx = np.asarray(W_ctx, dtype=np.float32)
    b_ctx = np.asarray(b_ctx, dtype=np.float32)
    start_token = np.asarray(start_token, dtype=np.float32)
    W_ih = np.asarray(W_ih, dtype=np.float32)
    b_ih = np.asarray(b_ih, dtype=np.float32)
    W_hh = np.asarray(W_hh, dtype=np.float32)
    b_hh = np.asarray(b_hh, dtype=np.float32)

    if "nc" not in _CACHE:
        _CACHE["nc"] = _build_nc()
    nc = _CACHE["nc"]

    ctxT = np.ascontiguousarray(np.concatenate([ws, gl], axis=1).T)
    ctxT_bf = ctxT.astype(bf16)
    wctx_bf = np.ascontiguousarray(W_ctx).astype(bf16)
    whh_bf = np.ascontiguousarray(W_hh).astype(bf16)
    wall_bf = np.ascontiguousarray(
        np.concatenate(
            [W_ih[:, : 2 * D] + W_hh[:, : 2 * D], W_ih[:, 2 * D :], W_hh[:, 2 * D :]],
            axis=1,
        )
    ).astype(bf16)
    gi1 = start_token @ W_ih + b_ih
    bias1 = np.ascontiguousarray(
        np.concatenate([gi1[: 2 * D] + b_hh[: 2 * D], b_hh[2 * D :]])
    ).astype(bf16)[None]
    gin1 = np.ascontiguousarray(gi1[2 * D :].astype(np.float32))[None]
    biasM = np.ascontiguousarray(
        np.concatenate([b_ih[: 2 * D] + b_hh[: 2 * D], b_ih[2 * D :], b_hh[2 * D :]])
    ).astype(bf16)[None]
    bctx = np.ascontiguousarray(b_ctx).astype(bf16)[None]

    shared = dict(
        wctx=wctx_bf, whh=whh_bf, wall=wall_bf, bctx=bctx,
        bias1=bias1, gin1=gin1, biasM=biasM,
    )
    in_maps = [
        {**shared, "ctxT": np.ascontiguousarray(ctxT_bf[:, i * BL : (i + 1) * BL])}
        for i in range(NCORES)
    ]

    LAST_IN_MAPS[0] = in_maps
    res = run_bass_kernel_spmd(
        nc, in_maps, core_ids=list(range(NCORES)), trace=TRACE, **TRACE_KW
    )
    LAST_RESULT[0] = res

    full = np.empty((B, T, D), dtype=np.float32)
    for i in range(NCORES):
        o = np.asarray(res.results[i]["out"]).astype(np.float32)
        full[i * BL : (i + 1) * BL] = o.transpose(1, 0, 2)
    return full
